# revision 1
# baseline (speedup 1.0000x reference)
"""Causal self-attention Trainium2 kernel (B=128, T=128, C=768, H=12, D=64).

Sharding: data-parallel over batch across 8 cores (16 batches/core).
Per-core pipeline (4-batch groups, feature-major activations):
  x -> PE-transpose -> x^T (fp32r)
  Q^T,K^T = W_qkv^T @ x^T  (fp32r matmuls, N=512)  -> fp16 tiles (K^T zero-padded)
  V       = x @ W_qkv[:,v] (token-major, fp32r, N=384) -> fp16 V' with ones col
  S^T_h   = Kz_h^T.T @ Q^T (fp16, K=128; +mask add via matmul)
  E^T     = exp(S^T * scale)  (ScalarE, fp16 out)
  O'_h    = E^T_h.T @ V'_h  (fp16, N=65: cols 0:64=O', col 64=rowsum)
  O       = O' * (1/rowsum)  (per-partition scalar, token-major, fp16)
  O^T via xbar DMA-transpose -> Y = O^T.T @ W_proj16 (fp16, N=384) -> DRAM
"""
import sys
import numpy as np

sys.path.insert(0, "/opt/trn_rl_repo")

import concourse.bass as bass  # noqa: E402
import concourse.tile as tile  # noqa: E402
from concourse import bacc, mybir  # noqa: E402
from concourse import bass_utils  # noqa: E402
from contextlib import ExitStack, nullcontext  # noqa: E402

F32 = mybir.dt.float32
F32R = mybir.dt.float32r
F16 = mybir.dt.float16

N_CORES = 8
B, T, C = 128, 128, 768
H, D = 12, 64
BC = B // N_CORES          # batches per core = 16
GB = 4                     # batches per group
NG = BC // GB              # groups per core = 4
GT = GB * T                # tokens per group = 512
NK = C // 128              # contraction k-tiles = 6
SCALE = D ** -0.5
MASKVAL = -30000.0
USE_XBAR_OT = False


def build_program(loop_iters=None):
    nc = bacc.Bacc("TRN2", target_bir_lowering=False, debug=False,
                   num_devices=N_CORES)
    x_d = nc.dram_tensor("x", [BC, T, C], F32, kind="ExternalInput").ap()
    wqkv_d = nc.dram_tensor("w_qkv", [C, 3 * C], F32, kind="ExternalInput").ap()
    wproj16_d = nc.dram_tensor("w_proj16", [C, C], F16, kind="ExternalInput").ap()
    mask_d = nc.dram_tensor("mask16", [128, T], F16, kind="ExternalInput").ap()
    ident16_d = nc.dram_tensor("ident16", [128, 128], F16, kind="ExternalInput").ap()
    ident32_d = nc.dram_tensor("ident32", [128, 128], F32, kind="ExternalInput").ap()
    y_d = nc.dram_tensor("y", [BC, T, C], F32, kind="ExternalOutput").ap()

    with tile.TileContext(nc) as tc, ExitStack() as ctx:
        cpool = ctx.enter_context(tc.tile_pool(name="const", bufs=1))
        gpool = ctx.enter_context(tc.tile_pool(name="grp", bufs=2))
        spool = ctx.enter_context(tc.tile_pool(name="small", bufs=4))
        pp = ctx.enter_context(tc.tile_pool(name="ps", bufs=1, space="PSUM"))

        # ---- constants / weights (resident) ----
        wqkv_r = cpool.tile([128, NK, 3 * C], F32R)
        for k in range(NK):
            nc.gpsimd.dma_start(wqkv_r[:, k, :],
                                wqkv_d.rearrange("(k p) f -> p k f", p=128)[:, k, :].bitcast(F32R))
        wproj16 = cpool.tile([128, NK, C], F16)
        nc.gpsimd.dma_start(wproj16, wproj16_d.rearrange("(k p) f -> p k f", p=128))
        mask16 = cpool.tile([128, T], F16)
        nc.sync.dma_start(mask16, mask_d)
        ident16 = cpool.tile([128, 128], F16)
        nc.sync.dma_start(ident16, ident16_d)
        ident32 = cpool.tile([128, 128], F32)
        nc.sync.dma_start(ident32, ident32_d)
        ii16 = ident16[:, None, :].broadcast_to([128, 2, 128])

        # persistent kz / vp (ping-pong): zero halves and ones cols written once
        kz_pp = [cpool.tile([128, H, GT], F16, name=f"kz{i}") for i in range(2)]
        vp_pp = [cpool.tile([128, GB, H, 65], F16, name=f"vp{i}") for i in range(2)]
        for kzt in kz_pp:
            nc.gpsimd.memset(kzt[64:128, 0:H:2, :], 0.0)
            nc.gpsimd.memset(kzt[0:64, 1:H:2, :], 0.0)
        for vpt in vp_pp:
            nc.gpsimd.memset(vpt[:, :, :, 64:65], 1.0)

        loop_cm = tc.For_i(0, loop_iters, 1) if loop_iters else nullcontext()
        with loop_cm:
            for g in range(NG):
                # ---- load x per batch + x^T via PE transpose ----
                xT = gpool.tile([128, NK, GB, 128], F32R, tag="xT", name=f"xT_{g}")
                for b in range(GB):
                    x_sb = gpool.tile([128, C], F32, tag="x_sb", bufs=3, name=f"x_sb_{g}_{b}")
                    nc.sync.dma_start(x_sb, x_d[g * GB + b].rearrange("t c -> t c"))
                    for k in range(NK):
                        xt_ps = pp.tile([128, 128], F32, tag="tps", bufs=1, name="xt_ps")
                        nc.tensor.transpose(xt_ps, x_sb[:, 128 * k:128 * (k + 1)], ident32)
                        nc.vector.tensor_copy(xT[:, k, b, :], xt_ps)

                # ---- Q^T / K^T projection (feature-major, fp32r, N=512) ----
                qT = gpool.tile([128, 6, GT], F16, tag="qT", name=f"qT_{g}")
                kz = kz_pp[g % 2]
                xg = xT.rearrange("p k b t -> p k (b t)")
                for f in range(12):
                    qk_ps = pp.tile([128, GT], F32, tag="qkps", bufs=2, name="qk_ps")
                    for k in range(NK):
                        nc.tensor.matmul(qk_ps, wqkv_r[:, k, 128 * f:128 * (f + 1)],
                                         xg[:, k, :], start=(k == 0), stop=(k == NK - 1))
                    if f < 6:
                        nc.scalar.copy(qT[:, f, :], qk_ps)
                    else:
                        h0 = 2 * (f - 6)
                        nc.vector.tensor_copy(kz[0:64, h0, :], qk_ps[0:64, :])
                        nc.vector.tensor_copy(kz[64:128, h0 + 1, :], qk_ps[64:128, :])

                # ---- V projection (token-major, fp32r, N=384) into V' ----
                vp = vp_pp[g % 2]
                for b in range(GB):
                    for half in range(2):
                        v_ps = pp.tile([128, 384], F32, tag="vps", bufs=2, name="v_ps")
                        for k in range(NK):
                            nc.tensor.matmul(
                                v_ps, xT[:, k, b, :],
                                wqkv_r[:, k, 2 * C + 384 * half:2 * C + 384 * (half + 1)],
                                start=(k == 0), stop=(k == NK - 1))
                        nc.scalar.copy(
                            vp[:, b, 6 * half:6 * (half + 1), 0:64],
                            v_ps.rearrange("p (h d) -> p h d", d=64))

                # ---- attention (token-major O, fp16) ----
                o_sb = gpool.tile([128, GB, C], F16, tag="o_sb", name=f"o_sb_{g}")
                ii4 = ident16[:, None, :].broadcast_to([128, 4, 128])
                for b in range(GB):
                    for dpr in range(3):
                        h0 = 4 * dpr
                        bs = slice(b * T, (b + 1) * T)
                        st_ps = pp.tile([128, 4, T], F32, tag="att", bufs=2, name="st_ps")
                        for j in range(4):
                            nc.tensor.matmul(st_ps[:, j, :], kz[:, h0 + j, bs],
                                             qT[:, 2 * dpr + j // 2, bs],
                                             start=(j == 0), stop=False)
                        nc.tensor.matmul(st_ps, mask16, ii4, start=False, stop=True)
                        eT = spool.tile([128, 4 * T], F16, tag="eT", name="eT")
                        nc.scalar.activation(eT, st_ps, mybir.ActivationFunctionType.Exp,
                                             scale=SCALE)
                        op_ps = pp.tile([128, 4, 65], F32, tag="att", bufs=2, name="op_ps")
                        for j in range(4):
                            nc.tensor.matmul(op_ps[:, j, :], eT[:, j * T:(j + 1) * T],
                                             vp[:, b, h0 + j, :],
                                             start=(j == 0), stop=(j == 3))
                        rinv = spool.tile([128, 4], F32, tag="rinv", name="rinv")
                        nc.vector.reciprocal(rinv, op_ps[:, :, 64])
                        nc.vector.tensor_tensor(
                            out=o_sb[:, b, h0 * D:(h0 + 4) * D].rearrange("p (h d) -> p h d", h=4),
                            in0=op_ps[:, :, 0:64],
                            in1=rinv[:, :, None].broadcast_to([128, 4, 64]),
                            op=mybir.AluOpType.mult)

                # ---- O^T via xbar DMA transpose, then Y projection (fp16) ----
                for b in range(GB):
                    oT = spool.tile([128, NK, 128], F16, tag="oT", bufs=2, name="oT")
                    if USE_XBAR_OT:
                        nc.sync.dma_start_transpose(oT, o_sb[:, b, :])
                    else:
                        for k in range(NK):
                            ot_ps = pp.tile([128, 128], F16, tag="otps", bufs=1, name="ot_ps")
                            nc.tensor.transpose(ot_ps, o_sb[:, b, 128 * k:128 * (k + 1)],
                                                ident16)
                            nc.vector.tensor_copy(oT[:, k, :], ot_ps)
                    y_sb = spool.tile([128, C], F32, tag="y_sb", bufs=2, name="y_sb")
                    for half in range(2):
                        y_ps = pp.tile([128, 384], F32, tag="vps", bufs=2, name="y_ps")
                        for k in range(NK):
                            nc.tensor.matmul(y_ps, oT[:, k, :],
                                             wproj16[:, k, 384 * half:384 * (half + 1)],
                                             start=(k == 0), stop=(k == NK - 1))
                        nc.scalar.copy(y_sb[:, 384 * half:384 * (half + 1)], y_ps)
                    nc.sync.dma_start(y_d[g * GB + b], y_sb)

    nc.compile()
    return nc


_PROGRAM = None
_in_maps_cache = None


def _host_consts():
    mask16 = np.where(np.arange(T)[None, :] <= np.arange(128)[:, None],
                      0.0, MASKVAL).astype(np.float16)
    ident16 = np.eye(128, dtype=np.float16)
    ident32 = np.eye(128, dtype=np.float32)
    return mask16, ident16, ident32


def make_in_maps(x, w_qkv, w_proj):
    x = np.ascontiguousarray(np.asarray(x), dtype=np.float32)
    w_qkv = np.ascontiguousarray(np.asarray(w_qkv), dtype=np.float32)
    w_proj16 = np.ascontiguousarray(np.asarray(w_proj), dtype=np.float16)
    mask16, ident16, ident32 = _host_consts()
    in_maps = []
    for c in range(N_CORES):
        in_maps.append({
            "x": x[c * BC:(c + 1) * BC],
            "w_qkv": w_qkv,
            "w_proj16": w_proj16,
            "mask16": mask16,
            "ident16": ident16,
            "ident32": ident32,
        })
    return in_maps


def kernel(x, w_qkv, w_proj):
    global _PROGRAM, _in_maps_cache
    if _PROGRAM is None:
        _PROGRAM = build_program()
    nc = _PROGRAM
    in_maps = make_in_maps(x, w_qkv, w_proj)
    _in_maps_cache = in_maps
    res = bass_utils.run_bass_kernel_spmd(nc, in_maps, core_ids=list(range(N_CORES)))
    out = np.concatenate([r["y"] for r in res.results], axis=0)
    return out.astype(np.float32)



# revision 30
# speedup vs baseline: 2.6731x; 2.6731x over previous
"""Causal self-attention Trainium2 kernel (B=128, T=128, C=768, H=12, D=64).

Sharding: data-parallel over batch across 8 cores (16 batches/core).
Per-core pipeline (4-batch groups, fp16 matmuls everywhere):
  x -> fp16 cast (Pool) -> x^T via XBAR DMA transpose
  Q^T,K^T = W_qkv^T @ x^T  (feature-major, N=512)  K^T -> zero-padded kz
  V       = x^T.T @ W_qkv[:,v] (token-major, N=384) -> V' with ones col
  S^T_h   = kz_h.T @ Q^T (fp16, K=128)
  E^T     = exp(S^T * scale) (Act) * causal01 (DVE)   [multiplicative mask]
  O'_h    = E^T_h.T @ V'_h  (N=65: cols 0:64=O', col 64=rowsum)
  O       = O' * (1/rowsum)  (DVE, token-major fp16)
  O^T via XBAR DMA transpose -> Y = O^T.T @ W_proj16 -> DMA from PSUM to DRAM
Attention units (b,dpr) software-pipelined with lag so PE never waits on
the Act-exp -> DVE-mask chain.
"""
import sys
import numpy as np

sys.path.insert(0, "/opt/trn_rl_repo")

import concourse.bass as bass  # noqa: E402
import concourse.tile as tile  # noqa: E402
from concourse import bacc, mybir  # noqa: E402
from concourse import bass_utils  # noqa: E402
from contextlib import ExitStack, nullcontext  # noqa: E402

F32 = mybir.dt.float32
F16 = mybir.dt.float16

N_CORES = 8
B, T, C = 128, 128, 768
H, D = 12, 64
BC = B // N_CORES          # batches per core = 16
GB = 4                     # batches per group
NG = BC // GB              # groups per core = 4
GT = GB * T                # tokens per group = 512
NK = C // 128              # contraction k-tiles = 6
SCALE = D ** -0.5
LAG = 4                    # attention software-pipeline depth (S -> EV)
LAG_Y = 2                  # further lag from o_sb complete to Y matmuls


def build_program(loop_iters=None, py_iters=1):
    nc = bacc.Bacc("TRN2", target_bir_lowering=False, debug=False,
                   num_devices=N_CORES)
    x_d = nc.dram_tensor("x", [BC, T, C], F32, kind="ExternalInput").ap()
    wqkv16_d = nc.dram_tensor("w_qkv16", [C, 3 * C], F16, kind="ExternalInput").ap()
    wproj16_d = nc.dram_tensor("w_proj16", [C, C], F16, kind="ExternalInput").ap()
    mask01_d = nc.dram_tensor("mask01", [128, T], F16, kind="ExternalInput").ap()
    ident16_d = nc.dram_tensor("ident16", [128, 128], F16, kind="ExternalInput").ap()
    y_d = nc.dram_tensor("y", [BC, T, C], F16, kind="ExternalOutput").ap()

    with tile.TileContext(nc) as tc, ExitStack() as ctx:
        cpool = ctx.enter_context(tc.tile_pool(name="const", bufs=1))
        gpool = ctx.enter_context(tc.tile_pool(name="grp", bufs=2))
        spool = ctx.enter_context(tc.tile_pool(name="small", bufs=4))
        pp = ctx.enter_context(tc.tile_pool(name="ps", bufs=1, space="PSUM"))

        # ---- constants / weights (resident) ----
        wqkv16 = cpool.tile([128, NK, 3 * C], F16)
        nc.gpsimd.dma_start(wqkv16, wqkv16_d.rearrange("(k p) f -> p k f", p=128))
        wproj16 = cpool.tile([128, NK, C], F16)
        nc.gpsimd.dma_start(wproj16, wproj16_d.rearrange("(k p) f -> p k f", p=128))
        mask01 = cpool.tile([128, T], F16)
        nc.sync.dma_start(mask01, mask01_d)
        ident16 = cpool.tile([128, 128], F16)
        nc.sync.dma_start(ident16, ident16_d)

        # persistent kz / vp (ping-pong): zero halves and ones cols written once
        kz_pp = [cpool.tile([128, H, GT], F16, name=f"kz{i}") for i in range(2)]
        vp_pp = [cpool.tile([128, GB, H, 65], F16, name=f"vp{i}") for i in range(2)]
        for kzt in kz_pp:
            nc.gpsimd.memset(kzt[64:128, 0:H:2, :], 0.0)
            nc.gpsimd.memset(kzt[0:64, 1:H:2, :], 0.0)
        for vpt in vp_pp:
            nc.gpsimd.memset(vpt[:, :, :, 64:65], 1.0)

        def emit_x_chain(g):
            """Load + cast + transpose x for group g; returns the xT tile."""
            xT = gpool.tile([128, NK, GB, 128], F16, tag="xT", name=f"xT_{g}")
            for b in range(GB):
                x_sb = gpool.tile([128, C], F32, tag="x_sb", bufs=5,
                                  name=f"x_sb_{g}_{b}")
                nc.sync.dma_start(x_sb, x_d[g * GB + b])
                x16 = gpool.tile([128, C], F16, tag="x16", bufs=5,
                                 name=f"x16_{g}_{b}")
                nc.gpsimd.tensor_copy(x16, x_sb)
                nc.sync.dma_start_transpose(xT[:, :, b, :], x16)
            return xT

        def emit_y(g, b, oT):
            """Y projection for batch b of group g, reading transposed oT."""
            y_sb = spool.tile([128, C], F16, tag="y_sb", bufs=4, name="y_sb")
            for half in range(2):
                y_ps = pp.tile([128, 384], F32, tag="vps", bufs=2, name="y_ps")
                for k in range(NK):
                    nc.tensor.matmul(y_ps, oT[:, k, :],
                                     wproj16[:, k, 384 * half:384 * (half + 1)],
                                     start=(k == 0), stop=(k == NK - 1))
                nc.vector.tensor_copy(y_sb[:, 384 * half:384 * (half + 1)], y_ps)
            nc.sync.dma_start(y_d[g * GB + b], y_sb)

        # prologue: first group's x-chain (steady state comes from the loop tail)
        xT_next = emit_x_chain(0)

        loop_cm = tc.For_i(0, loop_iters, 1) if loop_iters else nullcontext()
        with loop_cm:
          for _rep in range(py_iters):
            for g in range(NG):
                xT = xT_next
                # prefetch next group's x -> fp16 -> x^T (wraps to g=0 for the
                # next loop iteration; reloads the same data, which is benign)
                xT_next = emit_x_chain((g + 1) % NG)

                # ---- Q^T / K^T projection (feature-major, fp16, N=512) ----
                # deferred Y of the previous group's last 2 batches emitted
                # between early f-tiles to hide their O^T DMA latency.
                qT = gpool.tile([128, 6, GT], F16, tag="qT", name=f"qT_{g}")
                kz = kz_pp[g % 2]
                xg = xT.rearrange("p k b t -> p k (b t)")
                # interleave K (f>=6) and Q (f<6) so attention can start early
                for i, f in enumerate((6, 0, 7, 1, 8, 2, 9, 3, 10, 4, 11, 5)):
                    qk_ps = pp.tile([128, GT], F32, tag="big", bufs=4, name="qk_ps")
                    for k in range(NK):
                        nc.tensor.matmul(qk_ps, wqkv16[:, k, 128 * f:128 * (f + 1)],
                                         xg[:, k, :], start=(k == 0), stop=(k == NK - 1))
                    if f < 6:
                        nc.scalar.copy(qT[:, f, :], qk_ps)
                    else:
                        h0 = 2 * (f - 6)
                        # split the half-copies between DVE and Act
                        if f % 2 == 0:
                            nc.vector.tensor_copy(kz[0:64, h0, :], qk_ps[0:64, :])
                            nc.vector.tensor_copy(kz[64:128, h0 + 1, :],
                                                  qk_ps[64:128, :])
                        else:
                            nc.scalar.copy(kz[0:64, h0, :], qk_ps[0:64, :])
                            nc.scalar.copy(kz[64:128, h0 + 1, :],
                                           qk_ps[64:128, :])

                # ---- V projection (token-major, fp16, N=384) into V' ----
                vp = vp_pp[g % 2]
                for b in range(GB):
                    for half in range(2):
                        v_ps = pp.tile([128, 384], F32, tag="vps", bufs=2, name="v_ps")
                        for k in range(NK):
                            nc.tensor.matmul(
                                v_ps, xT[:, k, b, :],
                                wqkv16[:, k, 2 * C + 384 * half:2 * C + 384 * (half + 1)],
                                start=(k == 0), stop=(k == NK - 1))
                        nc.scalar.copy(
                            vp[:, b, 6 * half:6 * (half + 1), 0:64],
                            v_ps.rearrange("p (h d) -> p h d", d=64))

                # ---- attention + output, software-pipelined (12 units) ----
                o_sb = gpool.tile([128, GB, C], F16, tag="o_sb", name=f"o_sb_{g}")
                eTs = [None] * 12
                oTs = [None] * GB
                for step in range(12 + LAG + 3):
                    v = step - LAG
                    if 0 <= v < 12:
                        # EV first: frees the eT ring slot the mask below reuses
                        b, dpr = divmod(v, 3)
                        h0 = 4 * dpr
                        eT = eTs[v]
                        op_ps = pp.tile([128, 4, 65], F32, tag="op", bufs=2,
                                        name="op_ps")
                        for j in range(4):
                            nc.tensor.matmul(op_ps[:, j, :],
                                             eT[:, j, :],
                                             vp[:, b, h0 + j, :],
                                             start=(j == 0), stop=(j == 3))
                        rinv = spool.tile([128, 4], F32, tag="rinv", name="rinv")
                        nc.vector.reciprocal(rinv, op_ps[:, :, 64])
                        nc.vector.tensor_tensor(
                            out=o_sb[:, b, h0 * D:(h0 + 4) * D].rearrange(
                                "p (h d) -> p h d", h=4),
                            in0=op_ps[:, :, 0:64],
                            in1=rinv[:, :, None].broadcast_to([128, 4, 64]),
                            op=mybir.AluOpType.mult)
                        if dpr == 2:
                            # O^T via PE transposes (short latency vs XBAR DMA)
                            oT = spool.tile([128, NK, 128], F16, tag="oT",
                                            bufs=4, name="oT")
                            for hf in range(2):
                                ot_ps = pp.tile([128, 3, 128], F16, tag="op",
                                                bufs=2, name="ot_ps")
                                for k in range(3):
                                    kk = 3 * hf + k
                                    nc.tensor.transpose(
                                        ot_ps[:, k, :],
                                        o_sb[:, b, 128 * kk:128 * (kk + 1)],
                                        ident16)
                                nc.vector.tensor_copy(oT[:, 3 * hf:3 * hf + 3, :],
                                                      ot_ps)
                            oTs[b] = oT
                    if step < 12:
                        b, dpr = divmod(step, 3)
                        h0 = 4 * dpr
                        bs = slice(b * T, (b + 1) * T)
                        st_ps = pp.tile([128, 4, T], F32, tag="big", bufs=4,
                                        name="st_ps")
                        for j in range(4):
                            nc.tensor.matmul(st_ps[:, j, :], kz[:, h0 + j, bs],
                                             qT[:, 2 * dpr + j // 2, bs],
                                             start=(j == 0), stop=(j == 3))
                        e_raw = spool.tile([128, 4, T], F16, tag="e_raw", bufs=3,
                                           name="e_raw")
                        nc.scalar.activation(e_raw, st_ps,
                                             mybir.ActivationFunctionType.Exp,
                                             scale=SCALE)
                        eT = spool.tile([128, 4, T], F16, tag="eT", bufs=6,
                                        name="eT")
                        nc.vector.tensor_tensor(
                            out=eT, in0=e_raw,
                            in1=mask01[:, None, :].broadcast_to([128, 4, T]),
                            op=mybir.AluOpType.mult)
                        eTs[step] = eT
                    # Y once the oT copies have had LAG_Y units to land
                    w = step - LAG - LAG_Y
                    if 0 <= w < 12 and w % 3 == 2:
                        bb = w // 3
                        emit_y(g, bb, oTs[bb])

    nc.compile()
    return nc


_PROGRAM = None
_in_maps_cache = None


def _host_consts():
    # S^T layout: partition = key, column = query -> keep (key <= query)
    mask01 = np.where(np.arange(T)[None, :] >= np.arange(128)[:, None],
                      np.float16(1.0), np.float16(0.0)).astype(np.float16)
    ident16 = np.eye(128, dtype=np.float16)
    return mask01, ident16


def make_in_maps(x, w_qkv, w_proj):
    x = np.ascontiguousarray(np.asarray(x), dtype=np.float32)
    w_qkv16 = np.ascontiguousarray(np.asarray(w_qkv), dtype=np.float16)
    w_proj16 = np.ascontiguousarray(np.asarray(w_proj), dtype=np.float16)
    mask01, ident16 = _host_consts()
    in_maps = []
    for c in range(N_CORES):
        in_maps.append({
            "x": x[c * BC:(c + 1) * BC],
            "w_qkv16": w_qkv16,
            "w_proj16": w_proj16,
            "mask01": mask01,
            "ident16": ident16,
        })
    return in_maps


def kernel(x, w_qkv, w_proj):
    global _PROGRAM, _in_maps_cache
    if _PROGRAM is None:
        _PROGRAM = build_program()
    nc = _PROGRAM
    in_maps = make_in_maps(x, w_qkv, w_proj)
    _in_maps_cache = in_maps
    res = bass_utils.run_bass_kernel_spmd(nc, in_maps, core_ids=list(range(N_CORES)))
    out = np.concatenate([r["y"] for r in res.results], axis=0)
    return out.astype(np.float32)
